# revision 83
# baseline (speedup 1.0000x reference)
"""BiMamba block Trainium2 kernel (v3).

Sharding: data-parallel over (direction, batch) = 8 units, one per core.

v3 structure vs v2 (1579us -> 1379us in the concourse timeline sim):
  - Column-chunked front: P1a (in_proj xi cols 0:1024 + conv-a on DVE +
    silu) with x_proj accumulated per-block into a live psum; P1b (cols
    1024:2048) runs INSIDE the half-0 scan window on PE slack with conv-b
    as 4 diagonal matmuls on PE, split into a matmul phase (blks 2-9)
    and a silu+x_proj finisher (blks 11-12) so silus batch into one ACT
    table island; dt-b (softplus = Exp then Ln, e/ln grouped) at the H0
    tail (blks 13-15). dt-b must stay in H0: computing it in H1 races its
    stage write (scalar DMA queue) against the H1 stream read (sync
    queue) -- cross-queue DRAM write->read order is not sem-tracked.
  - z in_proj full-L inside half 0 (1-2 blocks per scan blk), silu taken
    straight from PSUM as one ACT island per block; yf for half 0 is
    finished inside half 0 (yf0 emitted at each block's end, which also
    frees the sz-a ring slot one block earlier for the late z silus), so
    out_proj gt0/gt1 run early in half 1.
  - B/C broadcast tiles allocated below xT in the pool stack so they
    survive into half 1 and are reloaded in place (no H1 warmup load).
  - D*xc enters psum_y via a diagonal matmul (starts the reduction);
    hc multiplies paired (2 states per [128, 2*HL] op).
  - Scans are DVE-only (neuronxcc rejects Pool scans); ALL hc pairs on
    DVE (any Pool op at the block tail chains into PE psum recycling),
    13 of 16 u-multiplies on Pool (U_DVE=3).
  - DRAM writes go on the scalar DGE queue, reads on the SP queue, so
    stage stream reads never sit behind staged writes (in-order queues).
  - PE p-state: per-block PE stream ordered [D-diag, DVE-pair idents,
    side jobs, Pool-pair idents] to avoid ramp-down bubbles.

Layout: channels on partitions (16 blocks of 128), time on free axis.
dA fp16 (scan state is fp32 internally); most tensors fp16.
"""

from contextlib import ExitStack

import numpy as np

D_MODEL, D_STATE, D_CONV = 1024, 16, 4
D_INNER = 2048
DT_RANK = 64
B_SZ, SEQ = 4, 2048
NB = D_INNER // 128  # 16 channel blocks
HL = SEQ // 2        # 1024
NPAIR = D_STATE // 2  # 8 hc pairs

# u-mult states on DVE (rest Pool); hc pairs on DVE (rest Pool)
U_DVE = 6
HC_DVE = 6

_CACHE = {}


def _pack_consts(conv_w, conv_b, dtb, Dp, A):
    # [128, 16*23] f32; per blk: A(16)|cw(4)|cb|dtb|Dp, rows = channel%128
    out = np.zeros((128, 16 * 23), np.float32)
    for blk in range(16):
        sl = slice(blk * 128, (blk + 1) * 128)
        out[:, blk * 23:blk * 23 + 16] = A[sl]
        out[:, blk * 23 + 16:blk * 23 + 20] = conv_w[sl]
        out[:, blk * 23 + 20] = conv_b[sl]
        out[:, blk * 23 + 21] = dtb[sl]
        out[:, blk * 23 + 22] = Dp[sl]
    return out


def _pad_xwT(xw):
    # xw: [96, 2048] -> transpose and pad to [2048, 112] with C at cols 96:112
    out = np.zeros((2048, 112), np.float16)
    xwT = xw.T.astype(np.float16)
    out[:, 0:80] = xwT[:, 0:80]
    out[:, 96:112] = xwT[:, 80:96]
    return out


def _pack_cdiag(conv_w):
    # [128, 16*4*128] f16: diag(conv_w[blk*128:(blk+1)*128, tap]) at col
    # block (blk*4+tap)
    out = np.zeros((128, 16 * 4 * 128), np.float16)
    idx = np.arange(128)
    for blk in range(16):
        for k in range(4):
            c0 = (blk * 4 + k) * 128
            out[idx, c0 + idx] = conv_w[blk * 128:(blk + 1) * 128, k]
    return out


def _pack_ddiag(Dp):
    # [128, 16*128] f16: diag(Dp[blk])
    out = np.zeros((128, 16 * 128), np.float16)
    idx = np.arange(128)
    for blk in range(16):
        out[idx, blk * 128 + idx] = Dp[blk * 128:(blk + 1) * 128]
    return out


def build_program():
    import concourse.bass as bass
    import concourse.bacc as bacc
    import concourse.tile as tile
    from concourse import mybir
    from concourse.masks import make_identity

    f16 = mybir.dt.float16
    f32 = mybir.dt.float32
    AF = mybir.ActivationFunctionType
    OP = mybir.AluOpType

    nc = bacc.Bacc()

    xT = nc.declare_dram_parameter("xT", [D_MODEL, SEQ], f16, isOutput=False)
    in_wT = nc.declare_dram_parameter("in_wT", [D_MODEL, 2 * D_INNER], f16, isOutput=False)
    XPW = 112  # x_proj out: dt_raw 0:64, B 64:80, pad, C 96:112
    xwT = nc.declare_dram_parameter("xwT", [D_INNER, XPW], f16, isOutput=False)
    dtwT = nc.declare_dram_parameter("dtwT", [DT_RANK, D_INNER], f16, isOutput=False)
    owT = nc.declare_dram_parameter("owT", [D_INNER, D_MODEL], f16, isOutput=False)
    CPW = 23
    consts_d = nc.declare_dram_parameter("consts_packed", [128, NB * CPW], f32, isOutput=False)
    cdiag_d = nc.declare_dram_parameter("cdiag", [128, NB * 4 * 128], f16, isOutput=False)
    ddiag_d = nc.declare_dram_parameter("ddiag", [128, NB * 128], f16, isOutput=False)
    y_out = nc.declare_dram_parameter("y", [D_MODEL, SEQ], f32, isOutput=True)

    # DRAM staging: stage_d rows pack xc(SEQ) | sz(SEQ) | dt(SEQ) | y1-H0(HL)
    stage_d = nc.dram_tensor("stage_d", [D_INNER, 3 * SEQ + HL], f16)
    yf_d = nc.dram_tensor("yf_d", [D_INNER, SEQ], f16)
    B_d = nc.dram_tensor("B_d", [D_STATE, SEQ], f16)
    C_d = nc.dram_tensor("C_d", [D_STATE, SEQ], f16)

    with tile.TileContext(nc) as tc, ExitStack() as ctx:
        consts = ctx.enter_context(tc.tile_pool(name="consts", bufs=1))

        I128 = consts.tile([128, 128], f16, tag="I128")
        make_identity(nc, I128)
        call = consts.tile([128, NB * CPW], f32, tag="call")
        nc.sync.dma_start(out=call, in_=consts_d[:, :])

        def A_col(blk, n):
            return call[:, blk * CPW + n:blk * CPW + n + 1]

        def cw_col(blk, k):
            return call[:, blk * CPW + 16 + k:blk * CPW + 16 + k + 1]

        def cb_col(blk):
            return call[:, blk * CPW + 20:blk * CPW + 21]

        def dtb_col(blk):
            return call[:, blk * CPW + 21:blk * CPW + 22]

        dtwT_sb = consts.tile([DT_RANK, D_INNER], f16, tag="dtwT")
        nc.scalar.dma_start(out=dtwT_sb, in_=dtwT[:, :])

        def dtw_lhsT(blk):
            return dtwT_sb[:, blk * 128:(blk + 1) * 128]
        dtraw = consts.tile([DT_RANK, HL], f16, tag="dtraw")
        carry_all = consts.tile([128, NB * D_STATE], f16, tag="carry_all")

        xitail = consts.tile([128, NB * 3], f16, tag="xitail")

        # bc0 sits below xT/xw in the pool stack: it survives H0 and is
        # reloaded in place for H1 (xT/xw release at H0 end above it).
        bc0_pool = tc.alloc_tile_pool(name="bc0", bufs=1)

        # xT resident until P1b (in_proj chunk-b rhs) completes mid-H0.
        xtpool = tc.alloc_tile_pool(name="xtpool", bufs=1)
        xT_sb = []
        for k in range(8):
            t = xtpool.tile([128, SEQ], f16, tag=f"xT{k}", name=f"xT{k}")
            nc.scalar.dma_start(out=t, in_=xT[k * 128:(k + 1) * 128, :])
            xT_sb.append(t)

        xwpool = tc.alloc_tile_pool(name="xwpool", bufs=1)  # released after P1b
        xwT_sb = []
        for k in range(NB):
            t = xwpool.tile([128, XPW], f16, tag=f"xwT{k}", name=f"xwT{k}")
            nc.scalar.dma_start(out=t, in_=xwT[k * 128:(k + 1) * 128, :])
            xwT_sb.append(t)

        # ---- FRONT: P1a (in_proj xi cols 0:HL + conv-a DVE + silu) with
        # x_proj-a accumulated per blk; then dtraw-a copies and P4a ----
        inwpool = tc.alloc_tile_pool(name="inwpool", bufs=1)  # released after front
        inw_sb = []
        for k in range(8):
            t = inwpool.tile([128, D_INNER], f16, tag=f"inw{k}")
            nc.scalar.dma_start(out=t, in_=in_wT[k * 128:(k + 1) * 128, 0:D_INNER])
            inw_sb.append(t)

        with tc.tile_pool(name="p1w", bufs=3) as p1w, \
             tc.tile_pool(name="p1pp", bufs=1, space="PSUM") as p1pp:
            psum_proj = p1pp.tile([XPW, HL], f32, tag="proja")
            p1ps = tc.alloc_tile_pool(name="p1ps", bufs=2, space="PSUM")
            pending = []
            for blk in range(NB):
                for fn in pending:
                    fn()
                pending = []
                psum = p1ps.tile([128, HL], f32, tag="xza")
                for nt in range(2):
                    cs = slice(nt * 512, (nt + 1) * 512)
                    for k in range(8):
                        nc.tensor.matmul(
                            psum[:, cs],
                            lhsT=inw_sb[k][:, blk * 128:(blk + 1) * 128],
                            rhs=xT_sb[k][:, cs],
                            start=(k == 0), stop=(k == 7),
                        )
                xi_t = p1w.tile([128, HL], f16, tag="xia")
                nc.scalar.activation(out=xi_t, in_=psum, func=AF.Copy)
                nc.vector.tensor_copy(
                    out=xitail[:, blk * 3:blk * 3 + 3], in_=xi_t[:, HL - 3:HL])
                # conv-a on DVE (v2 pattern; causal left edge = zero pad)
                acc = p1w.tile([128, HL], f16, tag="acca")
                nc.vector.tensor_scalar(
                    out=acc, in0=xi_t, scalar1=cw_col(blk, 3),
                    scalar2=cb_col(blk), op0=OP.mult, op1=OP.add,
                )
                for k in range(3):
                    d = 3 - k
                    tmp = p1w.tile([128, HL], f16, tag="cva")
                    nc.vector.tensor_scalar(
                        out=tmp, in0=xi_t, scalar1=cw_col(blk, k),
                        scalar2=None, op0=OP.mult,
                    )
                    nc.vector.tensor_tensor(
                        out=acc[:, d:], in0=tmp[:, :HL - d], in1=acc[:, d:],
                        op=OP.add,
                    )
                xc_t = p1w.tile([128, HL], f16, tag="xca")
                nc.scalar.activation(out=xc_t, in_=acc, func=AF.Silu)
                for nt in range(2):
                    cs = slice(nt * 512, (nt + 1) * 512)
                    nc.tensor.matmul(
                        psum_proj[:, cs], lhsT=xwT_sb[blk], rhs=xc_t[:, cs],
                        start=(blk == 0), stop=(blk == NB - 1),
                    )
                pending.append(lambda blk=blk, xc_t=xc_t: nc.sync.dma_start(
                    out=stage_d[blk * 128:(blk + 1) * 128, 0:HL], in_=xc_t))
            for fn in pending:
                fn()
            p1ps.release()
            # dtraw-a, B-a, C-a
            nc.scalar.activation(out=dtraw, in_=psum_proj[0:DT_RANK, :], func=AF.Copy)
            Ba = p1w.tile([D_STATE, HL], f16, tag="Ba")
            Ca = p1w.tile([D_STATE, HL], f16, tag="Ca")
            nc.scalar.activation(out=Ba, in_=psum_proj[64:80, :], func=AF.Copy)
            nc.scalar.activation(out=Ca, in_=psum_proj[96:112, :], func=AF.Copy)
            nc.sync.dma_start(out=B_d[:, 0:HL], in_=Ba)
            nc.sync.dma_start(out=C_d[:, 0:HL], in_=Ca)
            # start the H0 broadcast loads right away (before the stage
            # stream DMAs queue up behind them)
            B_bc0 = bc0_pool.tile([128, D_STATE * HL], f16, tag="B_bc")
            C_bc0 = bc0_pool.tile([128, D_STATE * HL], f16, tag="C_bc")
            for n0, n1 in ((0, 8), (8, D_STATE)):
                for tens, bc in ((B_d, B_bc0), (C_d, C_bc0)):
                    srcap = bass.AP(tensor=tens, offset=n0 * SEQ,
                                    ap=[[0, 128], [SEQ, n1 - n0], [1, HL]])
                    nc.sync.dma_start(out=bc[:, n0 * HL:n1 * HL], in_=srcap)
            # P4a: dt chunk a = softplus via Exp then Ln(1+e), batched by
            # function in groups of 8 to limit act-table switches
            p1pd = tc.alloc_tile_pool(name="p1pd", bufs=2, space="PSUM")
            dtp = []
            for g in range(2):
                e_t = []
                for j in range(8):
                    blk = g * 8 + j
                    psum_dt = p1pd.tile([128, HL], f32, tag="pdta")
                    for nt in range(2):
                        cs = slice(nt * 512, (nt + 1) * 512)
                        nc.tensor.matmul(
                            psum_dt[:, cs],
                            lhsT=dtw_lhsT(blk),
                            rhs=dtraw[:, cs], start=True, stop=True,
                        )
                    et = p1w.tile([128, HL], f16, tag="ea", bufs=8,
                                  name=f"ea{blk}")
                    nc.scalar.activation(
                        out=et, in_=psum_dt, func=AF.Exp,
                        bias=dtb_col(blk), scale=1.0)
                    e_t.append(et)
                for j in range(8):
                    blk = g * 8 + j
                    for fn in dtp:
                        fn()
                    dtp = []
                    dt_t = p1w.tile([128, HL], f16, tag="dta")
                    nc.scalar.activation(
                        out=dt_t, in_=e_t[j], func=AF.Ln, bias=1.0, scale=1.0)
                    dtp.append(lambda blk=blk, dt_t=dt_t: nc.sync.dma_start(
                        out=stage_d[blk * 128:(blk + 1) * 128,
                                    2 * SEQ:2 * SEQ + HL],
                        in_=dt_t))
            for fn in dtp:
                fn()
            p1pd.release()
        inwpool.release()

        owT_sb = []

        def emit_p6(gt, p6y, ppo):
            # out_proj for global 512-col tile gt; single merged yf load
            gs = slice(gt * 512, (gt + 1) * 512)
            # two half-gathers (k 0-7, 8-15) to halve the tile footprint
            yf_half = []
            for g in range(2):
                yh = p6y.tile([128, 8 * 512], f16, tag=f"yfall{g}",
                              name=f"yfall{g}_{gt}")
                ysrc = bass.AP(tensor=yf_d, offset=g * 8 * 128 * SEQ + gt * 512,
                               ap=[[SEQ, 128], [128 * SEQ, 8], [1, 512]])
                nc.sync.dma_start(out=yh, in_=ysrc)
                yf_half.append(yh)
            pend = []
            for m in range(8):
                psum_o = ppo.tile([128, 512], f32, tag="po", name=f"po{m}_{gt}")
                for k in range(NB):
                    nc.tensor.matmul(
                        psum_o, lhsT=owT_sb[k][:, m * 128:(m + 1) * 128],
                        rhs=yf_half[k // 8][:, (k % 8) * 512:(k % 8 + 1) * 512],
                        start=(k == 0), stop=(k == NB - 1),
                    )
                yo = p6y.tile([128, 512], f32, tag="yo", name=f"yo{m}_{gt}")
                nc.scalar.activation(out=yo, in_=psum_o, func=AF.Copy)
                for fn in pend:
                    fn()
                pend = [lambda m=m, yo=yo: nc.sync.dma_start(
                    out=y_out[m * 128:(m + 1) * 128, gs], in_=yo)]
            for fn in pend:
                fn()

        # ---------------- scan halves ----------------

        def emit_half(half, pools):
            (bc_pool, p5s, p5w, p5dA, ppy, p5u, p5hc, p5h, p5y1,
             extra) = pools
            hs = slice(half * HL, (half + 1) * HL)
            if extra.get("bc") is not None:
                B_bc, C_bc = extra["bc"]
            else:
                B_bc = bc_pool.tile([128, D_STATE * HL], f16, tag="B_bc")
                C_bc = bc_pool.tile([128, D_STATE * HL], f16, tag="C_bc")
                for n0, n1 in ((0, 8), (8, D_STATE)):
                    for tens, bc in ((B_d, B_bc), (C_d, C_bc)):
                        srcap = bass.AP(tensor=tens, offset=n0 * SEQ + half * HL,
                                        ap=[[0, 128], [SEQ, n1 - n0], [1, HL]])
                        nc.sync.dma_start(out=bc[:, n0 * HL:n1 * HL], in_=srcap)

            pending = []
            new_pending = []
            fifo = []
            PD = 2
            PITCH = 3 * SEQ + HL  # stage_d row pitch

            def load_stream(b):
                if half == 0:
                    # segs {xc-a @0, dt-a @2*SEQ}
                    cxd = p5s.tile([128, 2 * HL], f16, tag="cxd", bufs=2)
                    csrc = bass.AP(tensor=stage_d, offset=b * 128 * PITCH,
                                   ap=[[PITCH, 128], [2 * SEQ, 2], [1, HL]])
                    nc.sync.dma_start(out=cxd, in_=csrc)
                    xc_p, sz_p, dt_p = cxd[:, 0:HL], None, cxd[:, HL:2 * HL]
                else:
                    # segs {xc-b @HL, sz-b @SEQ+HL, dt-b @2*SEQ+HL}
                    cxd = p5s.tile([128, 3 * HL], f16, tag="cxd3", bufs=2)
                    csrc = bass.AP(tensor=stage_d, offset=b * 128 * PITCH + HL,
                                   ap=[[PITCH, 128], [SEQ, 3], [1, HL]])
                    nc.sync.dma_start(out=cxd, in_=csrc)
                    xc_p, sz_p, dt_p = (cxd[:, 0:HL], cxd[:, HL:2 * HL],
                                        cxd[:, 2 * HL:3 * HL])
                dd = p5w.tile([128, 128], f16, tag="dd", bufs=3)
                nc.sync.dma_start(out=dd, in_=ddiag_d[:, b * 128:(b + 1) * 128])
                dtxc = p5w.tile([128, HL], f16, tag="dtxc", bufs=3)
                nc.vector.tensor_tensor(out=dtxc, in0=dt_p, in1=xc_p, op=OP.mult)
                fifo.append((xc_p, sz_p, dt_p, dtxc, dd))

            for b in range(PD):
                load_stream(b)

            jobs = extra["jobs"]

            for blk in range(NB):
                rs = slice(blk * 128, (blk + 1) * 128)
                xc_t, sz_t, dt_h, dtxc, dd_t = fifo.pop(0)
                if blk + PD < NB:
                    load_stream(blk + PD)
                for fn in pending:
                    fn()
                pending = list(new_pending)
                new_pending.clear()

                # dA exps for all 16 n (ACT, exp table resident)
                dA_t = []
                for n in range(D_STATE):
                    dA = p5dA.tile([128, HL], f16, tag="dA", bufs=4)
                    nc.scalar.activation(
                        out=dA, in_=dt_h, func=AF.Exp, scale=A_col(blk, n))
                    dA_t.append(dA)
                # u multiplies: Pool states first (Pool runs ahead); DVE u's
                # are emitted inside the pair loop just before their scans
                # (same-engine ring: bufs=2 must not starve)
                u_t = [None] * D_STATE
                for n in range(U_DVE, D_STATE):
                    ns = slice(n * HL, (n + 1) * HL)
                    u = p5u.tile([128, HL], f16, tag="up",
                                     bufs=(4 if half == 0 else 7))
                    nc.gpsimd.tensor_tensor(out=u, in0=dtxc, in1=B_bc[:, ns], op=OP.mult)
                    u_t[n] = u
                psum_y = [ppy.tile([128, 512], f32, tag="py",
                                   bufs=(2 if half == 0 else 4),
                                   name=f"py{half}_{blk}_{i}") for i in range(2)]
                for nt in range(2):
                    cs = slice(nt * 512, (nt + 1) * 512)
                    nc.tensor.matmul(
                        psum_y[nt], lhsT=dd_t,
                        rhs=xc_t[:, cs], start=True, stop=False,
                    )

                def emit_pair(p):
                    n0, n1 = 2 * p, 2 * p + 1
                    for n in (n0, n1):
                        if n < U_DVE:
                            ns = slice(n * HL, (n + 1) * HL)
                            u = p5u.tile([128, HL], f16, tag="ud", bufs=2)
                            nc.vector.tensor_tensor(
                                out=u, in0=dtxc, in1=B_bc[:, ns], op=OP.mult)
                            u_t[n] = u
                    h_pair = p5h.tile([128, 2 * HL], f16, tag="hp", bufs=3)
                    for j, n in enumerate((n0, n1)):
                        cc = blk * D_STATE + n
                        init = 0.0 if half == 0 else carry_all[:, cc:cc + 1]
                        nc.vector.tensor_tensor_scan(
                            out=h_pair[:, j * HL:(j + 1) * HL],
                            data0=dA_t[n], data1=u_t[n], initial=init,
                            op0=OP.mult, op1=OP.add,
                        )
                        if half == 0:
                            nc.vector.tensor_copy(
                                out=carry_all[:, cc:cc + 1],
                                in_=h_pair[:, (j + 1) * HL - 1:(j + 1) * HL])
                    hc = p5hc.tile([128, 2 * HL], f16, tag="hc", bufs=2)
                    # C pair view: contiguous [n0 | n1] cols in C_bc
                    eng = nc.vector if p < HC_DVE else nc.gpsimd
                    eng.tensor_tensor(
                        out=hc, in0=h_pair, in1=C_bc[:, n0 * HL:(n1 + 1) * HL],
                        op=OP.mult)
                    for j in range(2):
                        for nt in range(2):
                            cs = slice(j * HL + nt * 512, j * HL + (nt + 1) * 512)
                            nc.tensor.matmul(
                                psum_y[nt], lhsT=I128, rhs=hc[:, cs],
                                start=False, stop=(p == NPAIR - 1 and j == 1),
                            )

                # DVE-produced hc pairs first (PE gets them promptly), then
                # the side jobs (PE/ACT filler, no hc dependency), then the
                # Pool-produced pairs — by which time Pool has caught up.
                for p in range(HC_DVE):
                    emit_pair(p)
                for job in jobs.get(blk, []):
                    job(new_pending)
                for p in range(HC_DVE, NPAIR):
                    emit_pair(p)
                if half == 0:
                    y1 = p5y1.tile([128, HL], f16, tag="y1", bufs=3,
                                   name=f"y1_{half}_{blk}")
                    for nt in range(2):
                        nc.scalar.activation(
                            out=y1[:, nt * 512:(nt + 1) * 512], in_=psum_y[nt],
                            func=AF.Copy)
                    extra["y1_tiles"][blk] = y1
                else:
                    # yf straight from the psum subtiles (1x TT, but keeps
                    # the copy off ACT)
                    yf = p5w.tile([128, HL], f16, tag="yf", bufs=2)
                    for nt in range(2):
                        cs = slice(nt * 512, (nt + 1) * 512)
                        nc.vector.tensor_tensor(
                            out=yf[:, cs], in0=psum_y[nt], in1=sz_t[:, cs],
                            op=OP.mult)
                    new_pending.append(lambda rs=rs, yf=yf: nc.sync.dma_start(
                        out=yf_d[rs, HL:SEQ], in_=yf))
            for fn in pending:
                fn()
            for fn in new_pending:
                fn()

        # ---- H0 with side jobs: z (full L) + silu-from-psum, P1b, P4b,
        # yf-H0 ----
        wstpool = tc.alloc_tile_pool(name="wst", bufs=2)
        cdpool = tc.alloc_tile_pool(name="cd", bufs=2)
        szapool = tc.alloc_tile_pool(name="sza", bufs=4)
        szbpool = tc.alloc_tile_pool(name="szb", bufs=2)
        xibpool = tc.alloc_tile_pool(name="xib", bufs=2)
        xcbpool = tc.alloc_tile_pool(name="xcb", bufs=2)
        # PSUM pools for H0 side jobs are phase-dynamic (8-bank budget with
        # ppy): ppsm (z + conv-b, [128,512]) lives all H0; ppb (in_proj-b /
        # dt-b [128,HL]) from blk 4; ppjb (x_proj-b) from blk 12.
        pp_hold = {"ppsm": tc.alloc_tile_pool(name="ppsm", bufs=2, space="PSUM")}
        psum_projb_holder = {}

        def job_alloc_ppb(new_pending):
            pp_hold["ppb"] = tc.alloc_tile_pool(name="ppb", bufs=1, space="PSUM")

        def job_alloc_ppjb(new_pending):
            pp_hold["ppjb"] = tc.alloc_tile_pool(name="ppjb", bufs=1, space="PSUM")

        sz_a = {}

        def job_z(zb, new_pending):
            # z in_proj full L for block zb; silu straight from psum — the 4
            # silus are one consecutive ACT island (2 table switches/blk).
            zw4 = wstpool.tile([128, 8 * 128], f16, tag="wst")
            zsrc = bass.AP(
                tensor=in_wT, offset=D_INNER + zb * 128,
                ap=[[2 * D_INNER, 128], [128 * 2 * D_INNER, 8], [1, 128]])
            nc.sync.dma_start(out=zw4, in_=zsrc)
            sza = szapool.tile([128, HL], f16, tag="sza", name=f"sza{zb}", bufs=4)
            szb = szbpool.tile([128, HL], f16, tag="szb", bufs=1)
            sz_a[zb] = sza
            for seg in range(4):
                psum_z = pp_hold["ppsm"].tile([128, 512], f32, tag="pz")
                for k in range(8):
                    nc.tensor.matmul(
                        psum_z, lhsT=zw4[:, k * 128:(k + 1) * 128],
                        rhs=xT_sb[k][:, seg * 512:(seg + 1) * 512],
                        start=(k == 0), stop=(k == 7),
                    )
                dst = sza if seg < 2 else szb
                ds = slice((seg % 2) * 512, (seg % 2) * 512 + 512)
                nc.scalar.activation(out=dst[:, ds], in_=psum_z, func=AF.Silu)
            new_pending.append(lambda zb=zb, szb=szb: nc.sync.dma_start(
                out=stage_d[zb * 128:(zb + 1) * 128, SEQ + HL:2 * SEQ], in_=szb))

        def job_yf0(j, new_pending):
            yf = szbpool.tile([128, HL], f16, tag="yf0", bufs=1)
            nc.vector.tensor_tensor(out=yf, in0=y1_h0[j], in1=sz_a[j], op=OP.mult)
            new_pending.append(lambda j=j, yf=yf: nc.sync.dma_start(
                out=yf_d[j * 128:(j + 1) * 128, 0:HL], in_=yf))

        def job_p1b_mm(pb, new_pending):
            # in_proj xi chunk b + conv-b (PE diag); raw conv+bias out
            # (Copy, bias fused) spilled to the stage xc-b slot. silu +
            # x_proj-b happen in job_p1b_fin.
            inb = wstpool.tile([128, 8 * 128], f16, tag="wst", name=f"inb{pb}")
            isrc = bass.AP(
                tensor=in_wT, offset=pb * 128,
                ap=[[2 * D_INNER, 128], [128 * 2 * D_INNER, 8], [1, 128]])
            nc.sync.dma_start(out=inb, in_=isrc)
            cdg = cdpool.tile([128, 4 * 128], f16, tag="cdg")
            nc.sync.dma_start(
                out=cdg, in_=cdiag_d[:, pb * 4 * 128:(pb + 1) * 4 * 128])
            psum_xzb = pp_hold["ppb"].tile([128, HL], f32, tag="xzb")
            for nt in range(2):
                cs = slice(nt * 512, (nt + 1) * 512)
                xcs = slice(HL + nt * 512, HL + (nt + 1) * 512)
                for k in range(8):
                    nc.tensor.matmul(
                        psum_xzb[:, cs], lhsT=inb[:, k * 128:(k + 1) * 128],
                        rhs=xT_sb[k][:, xcs], start=(k == 0), stop=(k == 7),
                    )
            xib = xibpool.tile([128, 3 + HL], f16, tag="xib", bufs=1)
            nc.vector.tensor_copy(
                out=xib[:, 0:3], in_=xitail[:, pb * 3:pb * 3 + 3])
            nc.scalar.activation(out=xib[:, 3:3 + HL], in_=psum_xzb, func=AF.Copy)
            cvb = xcbpool.tile([128, HL], f16, tag="cvb", name=f"cvb{pb}", bufs=2)
            for nt in range(2):
                psum_cb = pp_hold["ppsm"].tile([128, 512], f32, tag="pz", name=f"pcb{pb}_{nt}")
                for k in range(4):
                    nc.tensor.matmul(
                        psum_cb, lhsT=cdg[:, k * 128:(k + 1) * 128],
                        rhs=xib[:, k + nt * 512:k + nt * 512 + 512],
                        start=(k == 0), stop=(k == 3),
                    )
                nc.scalar.activation(
                    out=cvb[:, nt * 512:(nt + 1) * 512], in_=psum_cb,
                    func=AF.Copy)
            new_pending.append(lambda pb=pb, cvb=cvb: nc.sync.dma_start(
                out=stage_d[pb * 128:(pb + 1) * 128, HL:SEQ], in_=cvb))

        def job_p1b_fin(pb, new_pending):
            # restream raw conv-b, silu (batched at blks 12-13: one table
            # switch), x_proj-b accumulation, overwrite stage with silu'd xc
            cvr = xcbpool.tile([128, HL], f16, tag="cvb", name=f"cvr{pb}", bufs=2)
            nc.sync.dma_start(
                out=cvr, in_=stage_d[pb * 128:(pb + 1) * 128, HL:SEQ])
            xcb = xcbpool.tile([128, HL], f16, tag="xcb", name=f"xcb{pb}", bufs=2)
            nc.scalar.activation(out=xcb, in_=cvr, func=AF.Silu,
                                 bias=cb_col(pb), scale=1.0)
            if pb == 0:
                psum_projb_holder[0] = pp_hold["ppjb"].tile(
                    [XPW, HL], f32, tag="projb", name="projb")
            psum_projb = psum_projb_holder[0]
            for nt in range(2):
                cs = slice(nt * 512, (nt + 1) * 512)
                nc.tensor.matmul(
                    psum_projb[:, cs], lhsT=xwT_sb[pb], rhs=xcb[:, cs],
                    start=(pb == 0), stop=(pb == NB - 1),
                )
            new_pending.append(lambda pb=pb, xcb=xcb: nc.sync.dma_start(
                out=stage_d[pb * 128:(pb + 1) * 128, HL:SEQ], in_=xcb))
            if pb == NB - 1:
                nc.scalar.activation(
                    out=dtraw, in_=psum_projb[0:DT_RANK, :], func=AF.Copy)
                bcb = szbpool.tile([2 * D_STATE, HL], f16, tag="bcb", bufs=1)
                nc.scalar.activation(out=bcb[0:16, :], in_=psum_projb[64:80, :], func=AF.Copy)
                nc.scalar.activation(out=bcb[16:32, :], in_=psum_projb[96:112, :], func=AF.Copy)
                new_pending.append(lambda bcb=bcb: nc.sync.dma_start(
                    out=B_d[:, HL:SEQ], in_=bcb[0:16, :]))
                new_pending.append(lambda bcb=bcb: nc.sync.dma_start(
                    out=C_d[:, HL:SEQ], in_=bcb[16:32, :]))

        def job_p4b_group(dbs, new_pending, slot=2 * SEQ + HL,
                          psum_pool=None, sb_pool=None):
            # dt blocks: all Exp then all Ln (2 table switches total)
            e_ts = []
            for db in dbs:
                psum_dtb = (psum_pool or pp_hold["ppb"]).tile(
                    [128, HL], f32, tag="xzb", name=f"pdtb{db}")
                for nt in range(2):
                    cs = slice(nt * 512, (nt + 1) * 512)
                    nc.tensor.matmul(
                        psum_dtb[:, cs],
                        lhsT=dtw_lhsT(db),
                        rhs=dtraw[:, nt * 512:(nt + 1) * 512],
                        start=True, stop=True,
                    )
                e_t = (sb_pool or szbpool).tile([128, HL], f16, tag="dtb",
                                                bufs=3, name=f"eb{db}")
                nc.scalar.activation(
                    out=e_t, in_=psum_dtb, func=AF.Exp,
                    bias=dtb_col(db), scale=1.0)
                e_ts.append(e_t)
            for db, e_t in zip(dbs, e_ts):
                dtb_t = (sb_pool or szbpool).tile([128, HL], f16, tag="dtb",
                                                  bufs=3, name=f"dtb{db}")
                nc.scalar.activation(
                    out=dtb_t, in_=e_t, func=AF.Ln, bias=1.0, scale=1.0)
                new_pending.append(lambda db=db, dtb_t=dtb_t: nc.sync.dma_start(
                    out=stage_d[db * 128:(db + 1) * 128, 2 * SEQ + HL:3 * SEQ],
                    in_=dtb_t))

        def job_release_ppjb(new_pending):
            pp_hold["ppjb"].release()

        # H0 job schedule: z over blks 0-11 (2/2/2/2 then 1 each),
        # P1b matmuls blks 4-9 (3/blk), P1b silu+x_proj blks 10-11 (8 each),
        # P4b blks 12-15 (4 each, e/ln pairs spread thin).
        h0_jobs = {b: [] for b in range(NB)}
        for zb in range(16):
            b = zb // 2 if zb < 10 else zb - 5  # blks 0-4: 2 z's, 5-10: 1 z
            h0_jobs[b].append(lambda np_, zb=zb: job_z(zb, np_))
        h0_jobs[2].insert(0, job_alloc_ppb)
        for b in range(2, 10):
            for j in range(2):
                h0_jobs[b].append(lambda np_, pb=2 * (b - 2) + j: job_p1b_mm(pb, np_))
        h0_jobs[11].insert(0, job_alloc_ppjb)
        for b in (11, 12):
            for j in range(8):
                h0_jobs[b].append(lambda np_, pb=8 * (b - 11) + j: job_p1b_fin(pb, np_))
        h0_jobs[13].insert(0, job_release_ppjb)
        p4b_sched = [(13, 3), (14, 3), (15, 2)]
        gi = 0
        for b, cnt in p4b_sched:
            for _ in range(cnt):
                h0_jobs[b].append(
                    lambda np_, g=gi: job_p4b_group([2 * g, 2 * g + 1], np_))
                gi += 1


        p5s = tc.alloc_tile_pool(name="p5s0", bufs=2)
        p5w = tc.alloc_tile_pool(name="p5w0", bufs=3)
        p5u = tc.alloc_tile_pool(name="p5u0", bufs=7)
        p5hc = tc.alloc_tile_pool(name="p5hc0", bufs=4)
        p5h = tc.alloc_tile_pool(name="p5h0", bufs=4)
        p5y1 = tc.alloc_tile_pool(name="p5y10", bufs=4)
        p5dA = tc.alloc_tile_pool(name="p5dA0", bufs=3)
        ppy = tc.alloc_tile_pool(name="ppy0", bufs=2, space="PSUM")
        y1_h0 = {}
        emit_half(0, (None, p5s, p5w, p5dA, ppy, p5u, p5hc, p5h, p5y1,
                      {"jobs": h0_jobs, "bc": (B_bc0, C_bc0),
                       "y1_tiles": y1_h0}))
        # reload B/C broadcasts in place for H1 (waits H0's last readers)
        for n0, n1 in ((0, 8), (8, D_STATE)):
            for tens, bc in ((B_d, B_bc0), (C_d, C_bc0)):
                srcap = bass.AP(tensor=tens, offset=n0 * SEQ + HL,
                                ap=[[0, 128], [SEQ, n1 - n0], [1, HL]])
                nc.sync.dma_start(out=bc[:, n0 * HL:n1 * HL], in_=srcap)
        # release in reverse stack order (ppb was pushed mid-stream;
        # ppjb already released at blk 12)
        pp_hold["ppb"].release()
        ppy.release()
        p5dA.release()
        p5y1.release()
        p5h.release()
        p5hc.release()
        p5u.release()
        p5w.release()
        p5s.release()
        pp_hold["ppsm"].release()
        xcbpool.release()
        xibpool.release()
        szbpool.release()
        szapool.release()
        cdpool.release()
        wstpool.release()
        xwpool.release()
        xtpool.release()

        owpool = tc.alloc_tile_pool(name="owpool", bufs=1)
        for k in range(NB):
            t = owpool.tile([128, D_MODEL], f16, tag=f"owT{k}", name=f"owT{k}")
            nc.scalar.dma_start(out=t, in_=owT[k * 128:(k + 1) * 128, :])
            owT_sb.append(t)

        # H1: out_proj gt0 at blk1, gt1 at blk3; B/C reused from bc0
        with tc.tile_pool(name="p5s1", bufs=2) as p5s, \
             tc.tile_pool(name="p5w1", bufs=3) as p5w, \
             tc.tile_pool(name="p5u1", bufs=7) as p5u, \
             tc.tile_pool(name="p5hc1", bufs=4) as p5hc, \
             tc.tile_pool(name="p5h1", bufs=4) as p5h, \
             tc.tile_pool(name="p5y11", bufs=3) as p5y1, \
             tc.tile_pool(name="p5dA1", bufs=3) as p5dA, \
             tc.tile_pool(name="p6y", bufs=1) as p6y, \
             tc.tile_pool(name="ppy1", bufs=2, space="PSUM") as ppy, \
             tc.tile_pool(name="ppo", bufs=2, space="PSUM") as ppo:
            h1_jobs = {
                1: [lambda np_: emit_p6(0, p6y, ppo)],
                3: [lambda np_: emit_p6(1, p6y, ppo)],
            }
            emit_half(1, (None, p5s, p5w, p5dA, ppy, p5u, p5hc, p5h, p5y1,
                          {"jobs": h1_jobs, "y1_tiles": {},
                           "bc": (B_bc0, C_bc0)}))
            emit_p6(2, p6y, ppo)
            emit_p6(3, p6y, ppo)
        owpool.release()
        bc0_pool.release()

    nc.finalize()
    return nc


def _get_nc():
    if "nc" not in _CACHE:
        _CACHE["nc"] = build_program()
    return _CACHE["nc"]


def kernel(x, in_proj_w, conv_w, conv_b, x_proj_w, dt_proj_w, dt_proj_b,
           A_log, D_param, out_proj_w):
    from concourse.bass_utils import run_bass_kernel_spmd

    nc = _get_nc()

    x = np.asarray(x)
    wk = {}
    for d in range(2):
        cw = np.asarray(conv_w[d]).astype(np.float32)
        Dp = np.asarray(D_param[d]).astype(np.float32)
        wk[d] = {
            "in_wT": np.ascontiguousarray(np.asarray(in_proj_w[d]).T).astype(np.float16),
            "xwT": _pad_xwT(np.asarray(x_proj_w[d])),
            "dtwT": np.ascontiguousarray(np.asarray(dt_proj_w[d]).T).astype(np.float16),
            "owT": np.ascontiguousarray(np.asarray(out_proj_w[d]).T).astype(np.float16),
            "cdiag": _pack_cdiag(cw),
            "ddiag": _pack_ddiag(Dp),
            "consts_packed": _pack_consts(
                cw,
                np.asarray(conv_b[d]).astype(np.float32),
                np.asarray(dt_proj_b[d]).astype(np.float32),
                Dp,
                (-np.exp(np.asarray(A_log[d]))).astype(np.float32)),
        }

    in_maps = []
    for u in range(8):
        d, b = divmod(u, 4)
        xb = np.asarray(x[b])
        if d == 1:
            xb = xb[::-1]
        m = dict(wk[d])
        m["xT"] = np.ascontiguousarray(xb.T).astype(np.float16)
        in_maps.append(m)

    res = run_bass_kernel_spmd(nc, in_maps, core_ids=list(range(8))).results

    out = np.zeros((B_SZ, SEQ, D_MODEL), np.float32)
    for u in range(8):
        d, b = divmod(u, 4)
        yu = res[u]["y"].T  # [SEQ, D_MODEL]
        if d == 1:
            yu = yu[::-1]
        out[b] += yu
    return out.astype(np.float32)
